# revision 1
# baseline (speedup 1.0000x reference)
"""LoRA Linear (T=8192, D_in=D_out=4096, r=16) on 8 TRN2 NeuronCores.

out = x @ W^T + b + (32/16) * ((x_bf16 @ A^T) @ B^T)

Strategy: data-parallel over the 8192-token axis (1024 tokens/core).
Host pre-transposes operands so the contraction dim d lands on SBUF
partitions with perfectly contiguous DMA:
  xT  [4096, 1024] fp32  (per-core shard, SBUF-resident, stationary operand)
  WT  [4096, 4096] fp32  (replicated, streamed once per core, moving operand)
Base matmul runs as float32r (fp32 truncated to ~FP22 in the PE) which is
full-rate when the moving free dim >= 256 -- vs 4x slower true fp32.
LoRA: lora1^T = A @ x^T computed first (fp32r, rank-16 output), rounded to
bf16 (matching the reference's bf16 intermediate), then the rank-16
expansion matmul (bf16) seeds each PSUM accumulation group before the 32
base-matmul accumulations; bias is added on the PSUM->SBUF copy (DVE).
LoRA scaling (32/16 = 2.0) is folded into B^T on the host (exact in bf16).
"""

import numpy as np

try:
    import concourse  # noqa: F401
except ImportError:  # pragma: no cover
    import sys

    sys.path.insert(0, "/opt/trn_rl_repo")

from concourse import bacc, mybir, tile
from concourse.bass_utils import run_bass_kernel_spmd

N_CORES = 8
T, D_IN, D_OUT, R = 8192, 4096, 4096, 16
TPC = T // N_CORES  # 1024 tokens per core
N_DC = D_IN // 128  # 32 contraction chunks of 128
OC = 512  # output-column chunk (one PSUM bank of fp32)
N_OC = D_OUT // OC  # 8
N_TC = TPC // 128  # 8 token tiles of 128

f32 = mybir.dt.float32
f32r = mybir.dt.float32r
bf16 = mybir.dt.bfloat16

_NC_CACHE = {}


def build_nc(reps=1, loop_reps=0, w_once=False, x_bf16=False):
    xdt = bf16 if x_bf16 else f32r
    nc = bacc.Bacc(
        "TRN2", target_bir_lowering=False, debug=False, num_devices=N_CORES
    )
    xT = nc.dram_tensor("xT", [D_IN, TPC], xdt, kind="ExternalInput").ap()
    WT = nc.dram_tensor("WT", [D_IN, D_OUT], f32r, kind="ExternalInput").ap()
    AT = nc.dram_tensor("AT", [D_IN, R], xdt, kind="ExternalInput").ap()
    BT = nc.dram_tensor("BT", [R, D_OUT], bf16, kind="ExternalInput").ap()
    bias = nc.dram_tensor("bias", [128, D_OUT], f32, kind="ExternalInput").ap()
    out = nc.dram_tensor("out", [TPC, D_OUT], f32, kind="ExternalOutput").ap()

    with tile.TileContext(nc) as tc:
        with (
            tc.tile_pool(name="persist", bufs=1) as persist,
            tc.tile_pool(name="xpool", bufs=N_DC) as xpool,
            tc.tile_pool(name="wpool", bufs=4) as wpool,
            tc.tile_pool(name="opool", bufs=6) as opool,
            tc.tile_pool(name="pspool", bufs=8, space="PSUM") as pspool,
        ):
          def _emit_body():
            at_sb = persist.tile([128, N_DC * R], xdt, tag="at")
            bt_sb = persist.tile([R, D_OUT], bf16, tag="bt")
            bias_sb = persist.tile([128, D_OUT], f32, tag="bias")
            lora1_sb = persist.tile([R, TPC], bf16, tag="lora1")

            nc.sync.dma_start(out=bias_sb[:], in_=bias[:])
            nc.sync.dma_start(out=bt_sb[:], in_=BT[:])
            for dc in range(N_DC):
                nc.sync.dma_start(
                    out=at_sb[:, dc * R : (dc + 1) * R],
                    in_=AT[dc * 128 : (dc + 1) * 128, :],
                )

            xt_tiles = []
            for dc in range(N_DC):
                xt = xpool.tile([128, TPC], xdt, tag="xt")
                nc.sync.dma_start(
                    out=xt[:], in_=xT[dc * 128 : (dc + 1) * 128, :]
                )
                xt_tiles.append(xt)

            # Phase 1: lora1T[r, t] = sum_d A[r, d] * x[t, d]  (fp32r),
            # rounded to bf16 like the reference's bf16 einsum output.
            for th in range(TPC // OC):
                ps_l = pspool.tile([R, OC], f32, tag="ps")
                for dc in range(N_DC):
                    nc.tensor.matmul(
                        ps_l[:],
                        at_sb[:, dc * R : (dc + 1) * R],
                        xt_tiles[dc][:, th * OC : (th + 1) * OC],
                        start=(dc == 0),
                        stop=(dc == N_DC - 1),
                    )
                nc.vector.tensor_copy(
                    lora1_sb[:, th * OC : (th + 1) * OC], ps_l[:]
                )

            # Phase 2: out[t, o] = lora2 + sum_d x[t, d] W[o, d] + bias
            for oc in range(N_OC):
                osl = slice(oc * OC, (oc + 1) * OC)
                ps_tiles = [
                    pspool.tile([128, OC], f32, tag="ps", name=f"ps_{oc}_{t}")
                    for t in range(N_TC)
                ]
                # Seed each accumulation group with the rank-16 LoRA matmul.
                for t in range(N_TC):
                    nc.tensor.matmul(
                        ps_tiles[t][:],
                        lora1_sb[:, t * 128 : (t + 1) * 128],
                        bt_sb[:, osl],
                        start=True,
                        stop=False,
                    )
                if w_once:
                    wt0 = wpool.tile([128, OC], f32r, tag="wt", name=f"wto{oc}")
                    nc.sync.dma_start(out=wt0[:], in_=WT[0:128, osl])
                for dc in range(N_DC):
                    if w_once:
                        wt = wt0
                    else:
                        wt = wpool.tile([128, OC], f32r, tag="wt")
                        nc.sync.dma_start(
                            out=wt[:], in_=WT[dc * 128 : (dc + 1) * 128, osl]
                        )
                    for t in range(N_TC):
                        nc.tensor.matmul(
                            ps_tiles[t][:],
                            xt_tiles[dc][:, t * 128 : (t + 1) * 128],
                            wt[:],
                            start=False,
                            stop=(dc == N_DC - 1),
                        )
                for t in range(N_TC):
                    o_sb = opool.tile([128, OC], f32, tag="osb")
                    nc.vector.tensor_tensor(
                        o_sb[:],
                        ps_tiles[t][:],
                        bias_sb[:, osl],
                        mybir.AluOpType.add,
                    )
                    nc.sync.dma_start(
                        out=out[t * 128 : (t + 1) * 128, osl], in_=o_sb[:]
                    )

          if loop_reps:
              with tc.For_i(0, loop_reps, 1):
                  _emit_body()
          else:
              for _rep in range(reps):
                  _emit_body()

    nc.compile()
    return nc


def _prepare_in_maps(x, W, b, lora_a, lora_b, x_bf16=False, w_scale=1.0):
    import ml_dtypes

    xdt = ml_dtypes.bfloat16 if x_bf16 else np.float32
    WT = np.ascontiguousarray(W.T)  # [D_IN, D_OUT] fp32
    if w_scale != 1.0:
        WT = WT * np.float32(w_scale)
    AT = np.ascontiguousarray(lora_a.T).astype(xdt)  # [D_IN, R]
    # Fold the LoRA scaling (alpha/r = 2.0) into B^T; exact in bf16.
    BT = (np.ascontiguousarray(lora_b.T).astype(np.float32) * 2.0).astype(
        ml_dtypes.bfloat16
    )  # [R, D_OUT]
    bias = np.ascontiguousarray(
        np.broadcast_to(b.astype(np.float32), (128, D_OUT))
    )
    in_maps = []
    for c in range(N_CORES):
        xTc = np.ascontiguousarray(x[c * TPC : (c + 1) * TPC].T).astype(xdt)
        in_maps.append(
            {"xT": xTc, "WT": WT, "AT": AT, "BT": BT, "bias": bias}
        )
    return in_maps


def run(inputs, trace=False, **trace_kwargs):
    """Run on hardware; returns (full_output, BassKernelResults)."""
    if "nc" not in _NC_CACHE:
        _NC_CACHE["nc"] = build_nc()
    nc = _NC_CACHE["nc"]
    in_maps = _prepare_in_maps(
        np.asarray(inputs["x"], dtype=np.float32),
        np.asarray(inputs["W"], dtype=np.float32),
        np.asarray(inputs["b"], dtype=np.float32),
        np.asarray(inputs["lora_a"]),
        np.asarray(inputs["lora_b"]),
    )
    res = run_bass_kernel_spmd(
        nc, in_maps, list(range(N_CORES)), trace=trace, **trace_kwargs
    )
    out = np.concatenate(
        [res.results[c]["out"] for c in range(N_CORES)], axis=0
    )
    return out.astype(np.float32), res


def kernel(**inputs):
    out, _ = run(inputs, trace=False)
    return out


if __name__ == "__main__":
    rng = np.random.default_rng(0)
    import ml_dtypes

    x = rng.standard_normal((T, D_IN), dtype=np.float32)
    W = rng.standard_normal((D_OUT, D_IN), dtype=np.float32) * 0.02
    b = rng.standard_normal((D_OUT,), dtype=np.float32) * 0.02
    la = (rng.standard_normal((R, D_IN), dtype=np.float32) * 0.02).astype(
        ml_dtypes.bfloat16
    )
    lb = (rng.standard_normal((D_OUT, R), dtype=np.float32) * 0.02).astype(
        ml_dtypes.bfloat16
    )
    got = kernel(x=x, W=W, b=b, lora_a=la, lora_b=lb)
    ref = (
        x @ W.T
        + b
        + 2.0
        * (
            (x.astype(ml_dtypes.bfloat16).astype(np.float32) @ la.astype(np.float32).T)
            @ lb.astype(np.float32).T
        )
    )
    err = np.abs(got - ref).max() / np.abs(ref).max()
    print("scale-relative max err:", err)



# revision 2
# speedup vs baseline: 1.2319x; 1.2319x over previous
"""LoRA Linear (T=8192, D_in=D_out=4096, r=16) on 8 TRN2 NeuronCores.

out = x @ W^T + b + (32/16) * ((x_bf16 @ A^T) @ B^T)

Strategy: data-parallel over the 8192-token axis (1024 tokens/core).
The LoRA path is folded into the dense weight on the HOST:
    W' = W + 2.0 * (B @ A)   (fp32, exact)
so the device kernel is a single dense GEMM + bias. The reference's
bf16 LoRA intermediates differ from the exact fp32 fold by ~2e-4
relative -- far below the 2e-2 gate.

Device GEMM: per core out[1024, 4096] = xT'^T @ WT' + b with
  xT  [4096, 1024] fp16  (per-core shard, SBUF-resident, stationary)
  WT' [4096, 4096] fp16  (replicated, streamed, moving operand)
fp16 matmuls run at 1 col/cycle (same as fp32r) but halve HBM traffic
and enable fast weight load (FWL) for the stationary tiles.
DMA issue order is consumption order (x chunk dc + W chunk (0,dc)
interleaved) so the first matmul fires ~1us in and the PE HAM warms
once and stays warm.
"""

import numpy as np

try:
    import concourse  # noqa: F401
except ImportError:  # pragma: no cover
    import sys

    sys.path.insert(0, "/opt/trn_rl_repo")

from concourse import bacc, mybir, tile
from concourse.bass_utils import run_bass_kernel_spmd

N_CORES = 8
T, D_IN, D_OUT, R = 8192, 4096, 4096, 16
TPC = T // N_CORES  # 1024 tokens per core
N_DC = D_IN // 128  # 32 contraction chunks of 128
OC = 512  # output-column chunk (one PSUM bank of fp32)
N_OC = D_OUT // OC  # 8
N_TC = TPC // 128  # 8 token tiles of 128

f32 = mybir.dt.float32
f16 = mybir.dt.float16

_NC_CACHE = {}


def build_nc():
    nc = bacc.Bacc(
        "TRN2", target_bir_lowering=False, debug=False, num_devices=N_CORES
    )
    xT = nc.dram_tensor("xT", [D_IN, TPC], f16, kind="ExternalInput").ap()
    WT = nc.dram_tensor("WT", [D_IN, D_OUT], f16, kind="ExternalInput").ap()
    bias = nc.dram_tensor("bias", [128, D_OUT], f32, kind="ExternalInput").ap()
    out = nc.dram_tensor("out", [TPC, D_OUT], f32, kind="ExternalOutput").ap()

    with tile.TileContext(nc) as tc:
        with (
            tc.tile_pool(name="persist", bufs=1) as persist,
            tc.tile_pool(name="xpool", bufs=N_DC) as xpool,
            tc.tile_pool(name="wpool", bufs=6) as wpool,
            tc.tile_pool(name="opool", bufs=4) as opool,
            tc.tile_pool(name="pspool", bufs=8, space="PSUM") as pspool,
        ):
            bias_sb = persist.tile([128, D_OUT], f32, tag="bias")
            xt_tiles = [None] * N_DC

            for oc in range(N_OC):
                osl = slice(oc * OC, (oc + 1) * OC)
                ps_tiles = [
                    pspool.tile([128, OC], f32, tag="ps", name=f"ps_{oc}_{t}")
                    for t in range(N_TC)
                ]
                for dc in range(N_DC):
                    if oc == 0:
                        # JIT x load: issue each x chunk right before its
                        # first consumer so the rings stay in consumption
                        # order and the PE starts ~1us in.
                        xt = xpool.tile([128, TPC], f16, tag="xt")
                        nc.sync.dma_start(
                            out=xt[:], in_=xT[dc * 128 : (dc + 1) * 128, :]
                        )
                        xt_tiles[dc] = xt
                    wt = wpool.tile([128, OC], f16, tag="wt")
                    nc.sync.dma_start(
                        out=wt[:], in_=WT[dc * 128 : (dc + 1) * 128, osl]
                    )
                    if oc == 0 and dc == 0:
                        # bias is first needed at oc=0's copy-out (~55us
                        # in); issue after the first x/W chunks.
                        nc.sync.dma_start(out=bias_sb[:], in_=bias[:])
                    for t in range(N_TC):
                        nc.tensor.matmul(
                            ps_tiles[t][:],
                            xt_tiles[dc][:, t * 128 : (t + 1) * 128],
                            wt[:],
                            start=(dc == 0),
                            stop=(dc == N_DC - 1),
                        )
                for t in range(N_TC):
                    o_sb = opool.tile([128, OC], f32, tag="osb")
                    nc.vector.tensor_tensor(
                        o_sb[:],
                        ps_tiles[t][:],
                        bias_sb[:, osl],
                        mybir.AluOpType.add,
                    )
                    nc.sync.dma_start(
                        out=out[t * 128 : (t + 1) * 128, osl], in_=o_sb[:]
                    )

    nc.compile()
    return nc


def _prepare_in_maps(x, W, b, lora_a, lora_b):
    # Fold LoRA into the dense weight: W' = W + 2.0 * (B @ A), exact fp32.
    BA = lora_b.astype(np.float32) @ lora_a.astype(np.float32)
    Wp = W.astype(np.float32) + 2.0 * BA
    WT = np.ascontiguousarray(Wp.T).astype(np.float16)  # [D_IN, D_OUT]
    bias = np.ascontiguousarray(
        np.broadcast_to(b.astype(np.float32), (128, D_OUT))
    )
    in_maps = []
    for c in range(N_CORES):
        xTc = np.ascontiguousarray(
            x[c * TPC : (c + 1) * TPC].T, dtype=np.float16
        )
        in_maps.append({"xT": xTc, "WT": WT, "bias": bias})
    return in_maps


def run(inputs, trace=False, **trace_kwargs):
    """Run on hardware; returns (full_output, BassKernelResults)."""
    if "nc" not in _NC_CACHE:
        _NC_CACHE["nc"] = build_nc()
    nc = _NC_CACHE["nc"]
    in_maps = _prepare_in_maps(
        np.asarray(inputs["x"], dtype=np.float32),
        np.asarray(inputs["W"], dtype=np.float32),
        np.asarray(inputs["b"], dtype=np.float32),
        np.asarray(inputs["lora_a"]),
        np.asarray(inputs["lora_b"]),
    )
    res = run_bass_kernel_spmd(
        nc, in_maps, list(range(N_CORES)), trace=trace, **trace_kwargs
    )
    out = np.concatenate(
        [res.results[c]["out"] for c in range(N_CORES)], axis=0
    )
    return out.astype(np.float32), res


def kernel(**inputs):
    out, _ = run(inputs, trace=False)
    return out


if __name__ == "__main__":
    rng = np.random.default_rng(0)
    import ml_dtypes

    x = rng.standard_normal((T, D_IN), dtype=np.float32)
    W = rng.standard_normal((D_OUT, D_IN), dtype=np.float32) * 0.02
    b = rng.standard_normal((D_OUT,), dtype=np.float32) * 0.02
    la = (rng.standard_normal((R, D_IN), dtype=np.float32) * 0.02).astype(
        ml_dtypes.bfloat16
    )
    lb = (rng.standard_normal((D_OUT, R), dtype=np.float32) * 0.02).astype(
        ml_dtypes.bfloat16
    )
    got = kernel(x=x, W=W, b=b, lora_a=la, lora_b=lb)
    ref = (
        x @ W.T
        + b
        + 2.0
        * (
            (x.astype(ml_dtypes.bfloat16).astype(np.float32) @ la.astype(np.float32).T)
            @ lb.astype(np.float32).T
        )
    )
    err = np.abs(got - ref).max() / np.abs(ref).max()
    print("scale-relative max err:", err)


# revision 3
# speedup vs baseline: 1.2525x; 1.0167x over previous
"""LoRA Linear (T=8192, D_in=D_out=4096, r=16) on 8 TRN2 NeuronCores.

out = x @ W^T + b + (32/16) * ((x_bf16 @ A^T) @ B^T)

Strategy: data-parallel over the 8192-token axis (1024 tokens/core).
The LoRA path is folded into the dense weight on the HOST:
    W' = W + 2.0 * (B @ A)   (fp32, exact)
so the device kernel is a single dense GEMM + bias. The reference's
bf16 LoRA intermediates differ from the exact fp32 fold by ~2e-4
relative -- far below the 2e-2 gate.

Device GEMM per core: out[1024, 4096] = x^T W' + b, fp16 in / fp16 out
(PSUM accumulates fp32; host upcasts). fp16 streams 1 col/cycle like
fp32r but halves HBM traffic and enables FWL weight loads.

Schedule notes (from NTFF traces):
 - x and W are pre-interleaved on the host so every load is a single
   contiguous-per-partition 2D slice (4KB/2KB runs), halving the
   number of dma_start instructions (SP sequencer costs ~565ns each).
 - Loads issue on the SP queue, stores + bias on the Activation queue.
 - ~8 dummy matmuls on a zeroed tile run during the initial DMA wait
   so the PE HAM clock-gate warms (1.2->2.4 GHz) before real work.
 - PSUM: 8 banks = one oc block of 8 token tiles; DVE adds bias on the
   PSUM->SBUF copy with fp16 output.
"""

import numpy as np

try:
    import concourse  # noqa: F401
except ImportError:  # pragma: no cover
    import sys

    sys.path.insert(0, "/opt/trn_rl_repo")

from concourse import bacc, mybir, tile
from concourse.bass_utils import run_bass_kernel_spmd

N_CORES = 8
T, D_IN, D_OUT, R = 8192, 4096, 4096, 16
TPC = T // N_CORES  # 1024 tokens per core
N_DC2 = D_IN // 256  # 16 contraction chunk-pairs of 2x128
OC = 512  # output-column chunk (one PSUM bank of fp32)
N_OC = D_OUT // OC  # 8
N_TC = TPC // 128  # 8 token tiles of 128
N_WARM = 8  # HAM warmup matmuls

f32 = mybir.dt.float32
f16 = mybir.dt.float16

_NC_CACHE = {}


def build_nc():
    nc = bacc.Bacc(
        "TRN2", target_bir_lowering=False, debug=False, num_devices=N_CORES
    )
    # Host-interleaved layouts (see _prepare_in_maps):
    #   xT2[p, dc2*2048 + j*1024 + t] = x[t, dc2*256 + j*128 + p]
    #   WT2[p, ((oc*16)+dc2)*1024 + j*512 + o] = W'[oc*512 + o, dc2*256 + j*128 + p]
    xT2 = nc.dram_tensor("xT2", [128, N_DC2 * 2048], f16, kind="ExternalInput").ap()
    WT2 = nc.dram_tensor(
        "WT2", [128, N_OC * N_DC2 * 1024], f16, kind="ExternalInput"
    ).ap()
    bias = nc.dram_tensor("bias", [128, D_OUT], f32, kind="ExternalInput").ap()
    out = nc.dram_tensor("out", [TPC, D_OUT], f16, kind="ExternalOutput").ap()

    with tile.TileContext(nc) as tc:
        with (
            tc.tile_pool(name="persist", bufs=1) as persist,
            tc.tile_pool(name="xpool", bufs=N_DC2) as xpool,
            tc.tile_pool(name="wpool", bufs=8) as wpool,
            tc.tile_pool(name="opool", bufs=6) as opool,
            tc.tile_pool(name="pspool", bufs=8, space="PSUM") as pspool,
        ):
            # HAM warmup: zeroed operands, scratch PSUM bank, runs while
            # the first x/W DMAs are in flight.
            wz = persist.tile([128, OC], f16, tag="wz")
            nc.vector.memset(wz[:], 0.0)
            ps_warm = pspool.tile([128, OC], f32, tag="ps", name="ps_warm")
            for _ in range(N_WARM):
                nc.tensor.matmul(
                    ps_warm[:], wz[:, 0:128], wz[:], start=True, stop=True
                )

            bias_sb = persist.tile([128, D_OUT], f32, tag="bias")
            nc.scalar.dma_start(out=bias_sb[:], in_=bias[:])

            xt_tiles = [None] * N_DC2
            for oc in range(N_OC):
                osl = slice(oc * OC, (oc + 1) * OC)
                ps_tiles = [
                    pspool.tile([128, OC], f32, tag="ps", name=f"ps_{oc}_{t}")
                    for t in range(N_TC)
                ]
                for dc2 in range(N_DC2):
                    if oc == 0:
                        xt = xpool.tile([128, 2048], f16, tag="xt")
                        nc.sync.dma_start(
                            out=xt[:],
                            in_=xT2[:, dc2 * 2048 : (dc2 + 1) * 2048],
                        )
                        xt_tiles[dc2] = xt
                    wt = wpool.tile([128, 1024], f16, tag="wt")
                    wof = (oc * N_DC2 + dc2) * 1024
                    nc.sync.dma_start(out=wt[:], in_=WT2[:, wof : wof + 1024])
                    for j in range(2):
                        for t in range(N_TC):
                            nc.tensor.matmul(
                                ps_tiles[t][:],
                                xt_tiles[dc2][
                                    :, j * 1024 + t * 128 : j * 1024 + (t + 1) * 128
                                ],
                                wt[:, j * OC : (j + 1) * OC],
                                start=(dc2 == 0 and j == 0),
                                stop=(dc2 == N_DC2 - 1 and j == 1),
                            )
                for t in range(N_TC):
                    o_sb = opool.tile([128, OC], f16, tag="osb")
                    nc.vector.tensor_tensor(
                        o_sb[:],
                        ps_tiles[t][:],
                        bias_sb[:, osl],
                        mybir.AluOpType.add,
                    )
                    nc.scalar.dma_start(
                        out=out[t * 128 : (t + 1) * 128, osl], in_=o_sb[:]
                    )

    nc.compile()
    return nc


def _prepare_in_maps(x, W, b, lora_a, lora_b):
    # Fold LoRA into the dense weight: W' = W + 2.0 * (B @ A), exact fp32.
    BA = lora_b.astype(np.float32) @ lora_a.astype(np.float32)
    Wp = W.astype(np.float32) + 2.0 * BA

    # WT2[p, (oc, dc2, j, o)] = W'[oc*512+o, dc2*256+j*128+p]
    Wt = np.ascontiguousarray(Wp.T).astype(np.float16)  # [D_IN, D_OUT]
    W4 = Wt.reshape(N_DC2, 2, 128, N_OC, OC)  # [dc2, j, p, oc, o]
    WT2 = np.ascontiguousarray(
        W4.transpose(2, 3, 0, 1, 4).reshape(128, N_OC * N_DC2 * 1024)
    )

    bias = np.ascontiguousarray(
        np.broadcast_to(b.astype(np.float32), (128, D_OUT))
    )
    in_maps = []
    for c in range(N_CORES):
        xc = x[c * TPC : (c + 1) * TPC].T.astype(np.float16)  # [D_IN, TPC]
        x4 = xc.reshape(N_DC2, 2, 128, TPC)  # [dc2, j, p, t]
        xT2 = np.ascontiguousarray(
            x4.transpose(2, 0, 1, 3).reshape(128, N_DC2 * 2048)
        )
        in_maps.append({"xT2": xT2, "WT2": WT2, "bias": bias})
    return in_maps


def run(inputs, trace=False, **trace_kwargs):
    """Run on hardware; returns (full_output, BassKernelResults)."""
    if "nc" not in _NC_CACHE:
        _NC_CACHE["nc"] = build_nc()
    nc = _NC_CACHE["nc"]
    in_maps = _prepare_in_maps(
        np.asarray(inputs["x"], dtype=np.float32),
        np.asarray(inputs["W"], dtype=np.float32),
        np.asarray(inputs["b"], dtype=np.float32),
        np.asarray(inputs["lora_a"]),
        np.asarray(inputs["lora_b"]),
    )
    res = run_bass_kernel_spmd(
        nc, in_maps, list(range(N_CORES)), trace=trace, **trace_kwargs
    )
    out = np.concatenate(
        [res.results[c]["out"] for c in range(N_CORES)], axis=0
    )
    return out.astype(np.float32), res


def kernel(**inputs):
    out, _ = run(inputs, trace=False)
    return out


if __name__ == "__main__":
    rng = np.random.default_rng(0)
    import ml_dtypes

    x = rng.standard_normal((T, D_IN), dtype=np.float32)
    W = rng.standard_normal((D_OUT, D_IN), dtype=np.float32) * 0.02
    b = rng.standard_normal((D_OUT,), dtype=np.float32) * 0.02
    la = (rng.standard_normal((R, D_IN), dtype=np.float32) * 0.02).astype(
        ml_dtypes.bfloat16
    )
    lb = (rng.standard_normal((D_OUT, R), dtype=np.float32) * 0.02).astype(
        ml_dtypes.bfloat16
    )
    got = kernel(x=x, W=W, b=b, lora_a=la, lora_b=lb)
    ref = (
        x @ W.T
        + b
        + 2.0
        * (
            (x.astype(ml_dtypes.bfloat16).astype(np.float32) @ la.astype(np.float32).T)
            @ lb.astype(np.float32).T
        )
    )
    err = np.abs(got - ref).max() / np.abs(ref).max()
    print("scale-relative max err:", err)


# revision 6
# speedup vs baseline: 1.2741x; 1.0173x over previous
"""LoRA Linear (T=8192, D_in=D_out=4096, r=16) on 8 TRN2 NeuronCores.

out = x @ W^T + b + (32/16) * ((x_bf16 @ A^T) @ B^T)

Strategy: data-parallel over the 8192-token axis (1024 tokens/core).
The LoRA path is folded into the dense weight on the HOST:
    W' = W + 2.0 * (B @ A)   (fp32, exact)
so the device kernel is a single dense GEMM + bias. The reference's
bf16 LoRA intermediates differ from the exact fp32 fold by ~2e-4
relative -- far below the 2e-2 gate.

Device GEMM per core: out[1024, 4096] = x^T W' + b, fp16 in / fp16 out
(PSUM accumulates fp32; host upcasts). fp16 streams 1 col/cycle like
fp32r but halves HBM traffic and enables FWL weight loads.

Schedule notes (from NTFF traces):
 - x and W are pre-interleaved on the host so every load is a single
   contiguous-per-partition 2D slice (4KB/2KB runs), halving the
   number of dma_start instructions (SP sequencer costs ~565ns each).
 - Loads issue on the SP queue, stores + bias on the Activation queue.
 - ~8 dummy matmuls on a zeroed tile run during the initial DMA wait
   so the PE HAM clock-gate warms (1.2->2.4 GHz) before real work.
 - PSUM: 8 banks = one oc block of 8 token tiles; DVE adds bias on the
   PSUM->SBUF copy with fp16 output.
"""

import numpy as np

try:
    import concourse  # noqa: F401
except ImportError:  # pragma: no cover
    import sys

    sys.path.insert(0, "/opt/trn_rl_repo")

from concourse import bacc, mybir, tile
from concourse.bass_utils import run_bass_kernel_spmd

N_CORES = 8
T, D_IN, D_OUT, R = 8192, 4096, 4096, 16
TPC = T // N_CORES  # 1024 tokens per core
N_DC2 = D_IN // 256  # 16 contraction chunk-pairs of 2x128
OC = 512  # output-column chunk (one PSUM bank of fp32)
N_OC = D_OUT // OC  # 8
N_TC = TPC // 128  # 8 token tiles of 128
N_WARM = 16  # HAM warmup matmuls (bridge PE from engine-init to first data)

f32 = mybir.dt.float32
f16 = mybir.dt.float16

_NC_CACHE = {}


def build_nc():
    nc = bacc.Bacc(
        "TRN2", target_bir_lowering=False, debug=False, num_devices=N_CORES
    )
    # Host-interleaved layouts (see _prepare_in_maps):
    #   xT2[p, dc2*2048 + j*1024 + t] = x[t, dc2*256 + j*128 + p]
    #   WT2[p, ((oc*16)+dc2)*1024 + j*512 + o] = W'[oc*512 + o, dc2*256 + j*128 + p]
    xT2 = nc.dram_tensor("xT2", [128, N_DC2 * 2048], f16, kind="ExternalInput").ap()
    WT2 = nc.dram_tensor(
        "WT2", [128, N_OC * N_DC2 * 1024], f16, kind="ExternalInput"
    ).ap()
    # bias is DMA'd in per-oc [128, OC] slices: a single [128, 4096] fp32
    # transfer has 16KB/partition descriptors that round-robin 1:1 with
    # the 2KB W descriptors on the shared DMA engines and starve the
    # first W tile by ~6us (measured).
    bias = nc.dram_tensor("bias", [128, D_OUT], f32, kind="ExternalInput").ap()
    out = nc.dram_tensor("out", [TPC, D_OUT], f16, kind="ExternalOutput").ap()

    with tile.TileContext(nc) as tc:
        with (
            tc.tile_pool(name="persist", bufs=1) as persist,
            tc.tile_pool(name="xpool", bufs=N_DC2) as xpool,
            tc.tile_pool(name="wpool", bufs=24) as wpool,
            tc.tile_pool(name="bpool", bufs=3) as bpool,
            tc.tile_pool(name="opool", bufs=6) as opool,
            tc.tile_pool(name="pspool", bufs=8, space="PSUM") as pspool,
        ):
            # HAM warmup: zeroed operands, scratch PSUM bank, runs while
            # the first x/W DMAs are in flight so the PE clock-gate is
            # already at 2.4 GHz when real data lands.
            wz = persist.tile([128, OC], f16, tag="wz")
            nc.vector.memset(wz[:], 0.0)
            ps_warm = pspool.tile([128, OC], f32, tag="ps", name="ps_warm")
            for _ in range(N_WARM):
                nc.tensor.matmul(
                    ps_warm[:], wz[:, 0:128], wz[:], start=True, stop=True
                )

            xt_tiles = [None] * N_DC2

            def emit_mm(ps_tiles, dc2, j, t):
                nc.tensor.matmul(
                    ps_tiles[t][:],
                    xt_tiles[dc2][
                        :, j * 1024 + t * 128 : j * 1024 + (t + 1) * 128
                    ],
                    wt_tiles[dc2][:, j * OC : (j + 1) * OC],
                    start=(dc2 == 0 and j == 0),
                    stop=(dc2 == N_DC2 - 1 and j == 1),
                )

            def emit_copy_out(ps_tiles, bias_sb, oc, t):
                osl = slice(oc * OC, (oc + 1) * OC)
                o_sb = opool.tile([128, OC], f16, tag="osb")
                nc.vector.tensor_tensor(
                    o_sb[:],
                    ps_tiles[t][:],
                    bias_sb[:],
                    mybir.AluOpType.add,
                )
                nc.scalar.dma_start(
                    out=out[t * 128 : (t + 1) * 128, osl], in_=o_sb[:]
                )

            for oc in range(N_OC):
                ps_tiles = [
                    pspool.tile([128, OC], f32, tag="ps", name=f"ps_{oc}_{t}")
                    for t in range(N_TC)
                ]
                bias_sb = bpool.tile([128, OC], f32, tag="bias")
                nc.scalar.dma_start(
                    out=bias_sb[:], in_=bias[:, oc * OC : (oc + 1) * OC]
                )
                wt_tiles = [None] * N_DC2
                last = oc == N_OC - 1
                if not last:
                    # dc2-major: accumulate all 8 token tiles per W chunk,
                    # in x-arrival order (oc==0 loads x JIT).
                    for dc2 in range(N_DC2):
                        if oc == 0:
                            xt = xpool.tile([128, 2048], f16, tag="xt")
                            nc.sync.dma_start(
                                out=xt[:],
                                in_=xT2[:, dc2 * 2048 : (dc2 + 1) * 2048],
                            )
                            xt_tiles[dc2] = xt
                        wt = wpool.tile([128, 1024], f16, tag="wt")
                        wof = (oc * N_DC2 + dc2) * 1024
                        nc.sync.dma_start(
                            out=wt[:], in_=WT2[:, wof : wof + 1024]
                        )
                        wt_tiles[dc2] = wt
                        for j in range(2):
                            for t in range(N_TC):
                                emit_mm(ps_tiles, dc2, j, t)
                    for t in range(N_TC):
                        emit_copy_out(ps_tiles, bias_sb, oc, t)
                else:
                    # t-major last block: each token tile finishes its full
                    # accumulation first, so copies/stores overlap the
                    # remaining matmuls and the drain tail collapses to
                    # one tile's copy+store.
                    for dc2 in range(N_DC2):
                        wt = wpool.tile([128, 1024], f16, tag="wt")
                        wof = (oc * N_DC2 + dc2) * 1024
                        nc.sync.dma_start(
                            out=wt[:], in_=WT2[:, wof : wof + 1024]
                        )
                        wt_tiles[dc2] = wt
                    for t in range(N_TC):
                        for dc2 in range(N_DC2):
                            for j in range(2):
                                emit_mm(ps_tiles, dc2, j, t)
                        emit_copy_out(ps_tiles, bias_sb, oc, t)

    nc.compile()
    return nc


def _prepare_in_maps(x, W, b, lora_a, lora_b):
    # Fold LoRA into the dense weight: W' = W + 2.0 * (B @ A), exact fp32.
    BA = lora_b.astype(np.float32) @ lora_a.astype(np.float32)
    Wp = W.astype(np.float32) + 2.0 * BA

    # WT2[p, (oc, dc2, j, o)] = W'[oc*512+o, dc2*256+j*128+p]
    Wt = np.ascontiguousarray(Wp.T).astype(np.float16)  # [D_IN, D_OUT]
    W4 = Wt.reshape(N_DC2, 2, 128, N_OC, OC)  # [dc2, j, p, oc, o]
    WT2 = np.ascontiguousarray(
        W4.transpose(2, 3, 0, 1, 4).reshape(128, N_OC * N_DC2 * 1024)
    )

    bias = np.ascontiguousarray(
        np.broadcast_to(b.astype(np.float32), (128, D_OUT))
    )
    in_maps = []
    for c in range(N_CORES):
        xc = x[c * TPC : (c + 1) * TPC].T.astype(np.float16)  # [D_IN, TPC]
        x4 = xc.reshape(N_DC2, 2, 128, TPC)  # [dc2, j, p, t]
        xT2 = np.ascontiguousarray(
            x4.transpose(2, 0, 1, 3).reshape(128, N_DC2 * 2048)
        )
        in_maps.append({"xT2": xT2, "WT2": WT2, "bias": bias})
    return in_maps


def run(inputs, trace=False, **trace_kwargs):
    """Run on hardware; returns (full_output, BassKernelResults)."""
    if "nc" not in _NC_CACHE:
        _NC_CACHE["nc"] = build_nc()
    nc = _NC_CACHE["nc"]
    in_maps = _prepare_in_maps(
        np.asarray(inputs["x"], dtype=np.float32),
        np.asarray(inputs["W"], dtype=np.float32),
        np.asarray(inputs["b"], dtype=np.float32),
        np.asarray(inputs["lora_a"]),
        np.asarray(inputs["lora_b"]),
    )
    res = run_bass_kernel_spmd(
        nc, in_maps, list(range(N_CORES)), trace=trace, **trace_kwargs
    )
    out = np.concatenate(
        [res.results[c]["out"] for c in range(N_CORES)], axis=0
    )
    return out.astype(np.float32), res


def kernel(**inputs):
    out, _ = run(inputs, trace=False)
    return out


if __name__ == "__main__":
    rng = np.random.default_rng(0)
    import ml_dtypes

    x = rng.standard_normal((T, D_IN), dtype=np.float32)
    W = rng.standard_normal((D_OUT, D_IN), dtype=np.float32) * 0.02
    b = rng.standard_normal((D_OUT,), dtype=np.float32) * 0.02
    la = (rng.standard_normal((R, D_IN), dtype=np.float32) * 0.02).astype(
        ml_dtypes.bfloat16
    )
    lb = (rng.standard_normal((D_OUT, R), dtype=np.float32) * 0.02).astype(
        ml_dtypes.bfloat16
    )
    got = kernel(x=x, W=W, b=b, lora_a=la, lora_b=lb)
    ref = (
        x @ W.T
        + b
        + 2.0
        * (
            (x.astype(ml_dtypes.bfloat16).astype(np.float32) @ la.astype(np.float32).T)
            @ lb.astype(np.float32).T
        )
    )
    err = np.abs(got - ref).max() / np.abs(ref).max()
    print("scale-relative max err:", err)
